# revision 1
# baseline (speedup 1.0000x reference)
"""CrossSpatialAttention Trainium2 kernel.

Reference computation (per batch b, N = D*H*W = 8192 tokens, C=256, MID=64):
  f = relu(bn_f(Wf x)), g = relu(bn_g(Wg x)), h = Wh x          [MID, N]
  attn = softmax_m(f^T g / sqrt(MID))                            [N, N]
  z = attn @ h^T -> [MID, N];  out = Wv z + bv + x               [C, N]

Sharding: 8 cores = (batch b in {0,1}) x (query chunk of 2048 tokens).
Each core gets the full x for its batch (keys/values need all tokens) and
computes attention output for its 2048 queries.

Per-core kernel (Tile framework):
  - BN is folded into Wf/Wg + per-channel bias on host.
  - h/v biases commute through softmax (rows sum to 1), folded into a single
    per-output-channel bias bo = Wv @ bh + bv applied at the end.
  - S^T layout: for each key block m (128 keys), S^T[m, q] = (g_m)^T f via PE,
    exp on ScalarE (logits are >= 0 and <= ~14.1 for these inputs, so no
    max-subtraction is needed; exp and its 8192-term sum stay in fp32 range),
    then O^T[c, q] += [h^T | 1]_m @ P_m accumulated in PSUM -- the appended
    ones column produces the softmax denominator in row MID.
"""

import numpy as np

B, C, N = 2, 256, 8192
MID = 64
NCORES = 8
QC = N // 4            # queries per core (2048)
QT = 1024              # query tile (psum-sized)
EPS = 1e-5
SCALE = float(MID) ** -0.5
MB = 128               # key block
NMB = N // MB          # 64 key blocks

USE_F32R = True       # fp32r matmuls: 1 cyc/row (vs 4 for fp32) when N>=256

_cache = {}


def _build(loop_r=1):
    import concourse.bacc as bacc
    import concourse.tile as tile
    from concourse import mybir

    f32 = mybir.dt.float32
    f32r = mybir.dt.float32r
    AF = mybir.ActivationFunctionType

    fmm = f32r if USE_F32R else f32

    def mm(ap):
        return ap

    nc = bacc.Bacc(trn_type="TRN2", target_bir_lowering=False, debug=False)

    xb = nc.dram_tensor("xb", [C, N], f32, kind="ExternalInput").ap()
    xq = nc.dram_tensor("xq", [C, QC], f32, kind="ExternalInput").ap()
    wfT = nc.dram_tensor("wfT", [2, 128, MID], f32, kind="ExternalInput").ap()
    wgT = nc.dram_tensor("wgT", [2, 128, MID], f32, kind="ExternalInput").ap()
    whT = nc.dram_tensor("whT", [2, 128, MID], f32, kind="ExternalInput").ap()
    wvT = nc.dram_tensor("wvT", [MID, C], f32, kind="ExternalInput").ap()
    bf = nc.dram_tensor("bf", [MID, 1], f32, kind="ExternalInput").ap()
    bg = nc.dram_tensor("bg", [MID, 1], f32, kind="ExternalInput").ap()
    bo = nc.dram_tensor("bo", [C, 1], f32, kind="ExternalInput").ap()
    out = nc.dram_tensor("out", [C, QC], f32, kind="ExternalOutput").ap()

    with tile.TileContext(nc) as tc:
        with (
            tc.tile_pool(name="consts", bufs=1) as consts,
            tc.tile_pool(name="xpool", bufs=1) as xpool,
            tc.tile_pool(name="proj", bufs=1) as proj,
            tc.tile_pool(name="ppool", bufs=3) as ppool,
            tc.tile_pool(name="zpool", bufs=2) as zpool,
            tc.tile_pool(name="opool", bufs=3) as opool,
            tc.tile_pool(name="ps_st", bufs=2, space="PSUM") as ps_st,
            tc.tile_pool(name="ps_acc", bufs=1, space="PSUM") as ps_acc,
            tc.tile_pool(name="ps_gen", bufs=2, space="PSUM") as ps_gen,
        ):
            import contextlib
            loop_ctx = (tc.For_i(0, loop_r, 1) if loop_r > 1
                        else contextlib.nullcontext())
            with loop_ctx:
                # ---- constants ----
                wf_t = consts.tile([128, 2, MID], fmm)
                wg_t = consts.tile([128, 2, MID], fmm)
                wh_t = consts.tile([128, 2, MID], fmm)
                wv_t = consts.tile([MID, C], fmm)
                bf_t = consts.tile([MID, 1], f32)
                bg_t = consts.tile([MID, 1], f32)
                bo_t = consts.tile([128, C // 128, 1], f32)
                nc.gpsimd.dma_start(out=wf_t, in_=wfT.rearrange("k p m -> p k m").bitcast(fmm))
                nc.gpsimd.dma_start(out=wg_t, in_=wgT.rearrange("k p m -> p k m").bitcast(fmm))
                nc.gpsimd.dma_start(out=wh_t, in_=whT.rearrange("k p m -> p k m").bitcast(fmm))
                nc.gpsimd.dma_start(out=wv_t, in_=wvT.bitcast(fmm))
                nc.gpsimd.dma_start(out=bf_t, in_=bf)
                nc.gpsimd.dma_start(out=bg_t, in_=bg)
                nc.gpsimd.dma_start(out=bo_t, in_=bo.rearrange("(o p) x -> p o x", p=128))

                # ---- x tiles: full batch [128, 2, N], queries [128, 2, QC] ----
                x_t = xpool.tile([128, 2, N], fmm)
                xq_t = xpool.tile([128, 2, QC], fmm)
                xb_r = xb.bitcast(fmm).rearrange("(k p) n -> k p n", p=128)
                xq_r = xq.bitcast(fmm).rearrange("(k p) n -> k p n", p=128)
                for k in range(2):
                    for half in range(2):
                        sl = slice(half * (N // 2), (half + 1) * (N // 2))
                        nc.sync.dma_start(out=x_t[:, k, sl], in_=xb_r[k, :, sl])
                    nc.sync.dma_start(out=xq_t[:, k, :], in_=xq_r[k, :, :])

                # ---- priming matmuls: absorb each DMA semaphore into PE's
                # observed clock so no later matmul needs >1 sync wait (walrus
                # caps fused-matmul waits at 1) ----
                prime_srcs = [wf_t[:, 0, :], wg_t[:, 0, :], wh_t[:, 0, :],
                              wv_t[:, 0:MID]]
                for k in range(2):
                    for half in range(2):
                        o = half * (N // 2)
                        prime_srcs.append(x_t[:, k, o:o + MID])
                    prime_srcs.append(xq_t[:, k, 0:MID])
                dp = ps_st.tile([128, QT], f32, tag="st")
                for i, src in enumerate(prime_srcs):
                    nc.tensor.matmul(dp[0:src.shape[-1], 0:MID], src, src[:, 0:MID],
                                     start=(i == 0), stop=(i == len(prime_srcs) - 1),
                                     skip_group_check=True)
                trash = ppool.tile([128, QT], f32, tag="p")
                nc.scalar.activation(trash, dp, AF.Copy, bias=0.0, scale=1.0)

                # ---- projections ----
                g_t = proj.tile([MID, N], fmm)
                f_t = proj.tile([MID, QC], fmm)
                hTo = proj.tile([128, NMB, MID + 1], fmm)  # [m, block, c|1]

                # ones column via ScalarE so every hTo producer is on ACT
                nc.scalar.activation(hTo[:, :, MID], wg_t[:, 0, :],
                                     AF.Copy, bias=1.0, scale=0.0)

                # g = relu(Wg' x + bg'), full N
                for n in range(N // 512):
                    sl = slice(n * 512, (n + 1) * 512)
                    pg = ps_gen.tile([MID, 512], f32, tag="pg")
                    for k in range(2):
                        nc.tensor.matmul(pg, mm(wg_t[:, k, :]), mm(x_t[:, k, sl]),
                                         start=(k == 0), stop=(k == 1))
                    nc.scalar.activation(g_t[:, sl], pg, AF.Relu, bias=bg_t, scale=1.0)
                # f = relu(Wf' xq + bf'), QC queries
                for n in range(QC // 512):
                    sl = slice(n * 512, (n + 1) * 512)
                    pf = ps_gen.tile([MID, 512], f32, tag="pg")
                    for k in range(2):
                        nc.tensor.matmul(pf, mm(wf_t[:, k, :]), mm(xq_t[:, k, sl]),
                                         start=(k == 0), stop=(k == 1))
                    nc.scalar.activation(f_t[:, sl], pf, AF.Relu, bias=bf_t, scale=1.0)
                # hT[m, c] = x^T Wh^T, one accumulation region per psum tile
                for mb in range(NMB):
                    ph = ps_gen.tile([128, MID], f32, tag="pg")
                    msl = slice(mb * MB, (mb + 1) * MB)
                    for k in range(2):
                        nc.tensor.matmul(ph, mm(x_t[:, k, msl]), mm(wh_t[:, k, :]),
                                         start=(k == 0), stop=(k == 1))
                    nc.scalar.activation(hTo[:, mb, 0:MID], ph, AF.Copy,
                                         bias=0.0, scale=1.0)

                # ---- attention ----
                for qi in range(QC // QT):
                    qsl = slice(qi * QT, (qi + 1) * QT)
                    o_ps = ps_acc.tile([MID + 1, QT], f32, tag="acc")
                    for mb in range(NMB):
                        msl = slice(mb * MB, (mb + 1) * MB)
                        st = ps_st.tile([128, QT], f32, tag="st")
                        for h in range(QT // 512):
                            fs = slice(qi * QT + h * 512, qi * QT + (h + 1) * 512)
                            nc.tensor.matmul(st[:, h * 512:(h + 1) * 512],
                                             mm(g_t[:, msl]), mm(f_t[:, fs]),
                                             start=True, stop=True)
                        p_t = ppool.tile([128, QT], fmm, tag="p")
                        nc.scalar.activation(p_t, st, AF.Exp, scale=SCALE)
                        for h in range(QT // 512):
                            hs = slice(h * 512, (h + 1) * 512)
                            nc.tensor.matmul(o_ps[:, hs], mm(hTo[:, mb, :]),
                                             mm(p_t[:, hs]),
                                             start=(mb == 0), stop=(mb == NMB - 1))

                    # normalize: z = O / denom
                    rd = zpool.tile([1, QT], f32, tag="rd")
                    nc.vector.reciprocal(rd, o_ps[MID:MID + 1, :])
                    rb = zpool.tile([MID, QT], f32, tag="rb")
                    nc.gpsimd.partition_broadcast(rb, rd)
                    z_t = zpool.tile([MID, QT], fmm, tag="z")
                    nc.vector.tensor_mul(z_t, o_ps[0:MID, :], rb)

                    # out = Wv z + bo + xq
                    for oh in range(C // 128):
                        osl = slice(oh * 128, (oh + 1) * 128)
                        for h in range(QT // 512):
                            hs = slice(h * 512, (h + 1) * 512)
                            po = ps_gen.tile([128, 512], f32, tag="pg")
                            nc.tensor.matmul(po, mm(wv_t[:, osl]), mm(z_t[:, hs]),
                                             start=True, stop=True)
                            o_sb = opool.tile([128, 512], f32, tag="ob")
                            nc.vector.tensor_scalar_add(o_sb, po, bo_t[:, oh, :])
                            qs = slice(qi * QT + h * 512, qi * QT + (h + 1) * 512)
                            nc.vector.tensor_add(o_sb, o_sb, xq_t[:, oh, qs].bitcast(f32))
                            nc.sync.dma_start(out=out.rearrange(
                                "(o p) n -> o p n", p=128)[oh, :, qs], in_=o_sb)

    nc.compile()
    return nc


def _prep_inputs(inputs):
    f32 = np.float32
    x = np.asarray(inputs["x"], f32).reshape(B, C, N)

    def fold(W, b, gam, bet, m, v):
        inv = np.asarray(gam, f32) / np.sqrt(np.asarray(v, f32) + EPS)
        We = np.asarray(W, f32) * inv[:, None]
        be = np.asarray(b, f32) * inv + np.asarray(bet, f32) - np.asarray(m, f32) * inv
        return We, be

    Wf, bfe = fold(inputs["Wf"], inputs["bf"], inputs["gamf"], inputs["betf"],
                   inputs["mf"], inputs["vf"])
    Wg, bge = fold(inputs["Wg"], inputs["bg"], inputs["gamg"], inputs["betg"],
                   inputs["mg"], inputs["vg"])
    Wh = np.asarray(inputs["Wh"], f32)
    Wv = np.asarray(inputs["Wv"], f32)
    bo = Wv @ np.asarray(inputs["bh"], f32) + np.asarray(inputs["bv"], f32)

    wfT = np.ascontiguousarray(Wf.T.reshape(2, 128, MID))
    wgT = np.ascontiguousarray(Wg.T.reshape(2, 128, MID))
    whT = np.ascontiguousarray(Wh.T.reshape(2, 128, MID))
    wvT = np.ascontiguousarray(Wv.T)

    in_maps = []
    for core in range(NCORES):
        b, qc = divmod(core, 4)
        in_maps.append({
            "xb": np.ascontiguousarray(x[b]),
            "xq": np.ascontiguousarray(x[b][:, qc * QC:(qc + 1) * QC]),
            "wfT": wfT, "wgT": wgT, "whT": whT, "wvT": wvT,
            "bf": bfe.reshape(MID, 1).copy(),
            "bg": bge.reshape(MID, 1).copy(),
            "bo": bo.reshape(C, 1).copy(),
        })
    return in_maps


def _run(inputs, trace=False, **kw):
    from concourse.bass_utils import run_bass_kernel_spmd

    if "nc" not in _cache:
        _cache["nc"] = _build()
    in_maps = _prep_inputs(inputs)
    br = run_bass_kernel_spmd(_cache["nc"], in_maps, list(range(NCORES)),
                              trace=trace, **kw)
    out = np.empty((B, C, N), np.float32)
    for core in range(NCORES):
        b, qc = divmod(core, 4)
        out[b][:, qc * QC:(qc + 1) * QC] = br.results[core]["out"]
    return out.reshape(B, C, 8, 32, 32), br


def kernel(**inputs):
    out, _ = _run(inputs)
    return out



# revision 35
# speedup vs baseline: 39.1289x; 39.1289x over previous
"""CrossSpatialAttention Trainium2 kernel.

Reference computation (per batch b, N = D*H*W = 8192 tokens, C=256, MID=64):
  f = relu(bn_f(Wf x)), g = relu(bn_g(Wg x)), h = Wh x          [MID, N]
  attn = softmax_m(f^T g / sqrt(MID))                            [N, N]
  z = attn @ h^T -> [MID, N];  out = Wv z + bv + x               [C, N]

Sharding: 8 cores = (batch b in {0,1}) x (query chunk of 2048 tokens).
Each core gets the full x for its batch with its OWN query chunk rotated to
the front columns (softmax over keys is permutation-invariant, so rotating
the key order changes nothing). Queries are then just x[:, 0:QC] — no
separate query DMA.

Engine budget per core (cost-model):
  ACT: 128 x exp([128,1024]) ~ 141 us  <- bottleneck; nothing else on ACT
  PE : S^T + PV + projections ~ 134 us
  DVE: proj bias+relu (tensor_scalar), hT copies, normalize, out fuse ~ 45 us
  Pool: weight DMAs, ones memset, partition_broadcast
  DMA: x split in 10 chunks over 4 queues so projections start ~2 us in.

Numerics: BN folded into Wf/Wg + bias on host; h/v biases commute through
softmax (rows sum to 1) and fold into bo = Wv bh + bv. exp needs no
max-subtraction (logits in [0, ~14.1], fits fp32). The ones column appended
to hT yields the softmax denominator in PSUM row MID.
"""

import numpy as np

B, C, N = 2, 256, 8192
MID = 64
NCORES = 8
QC = N // 4            # queries per core (2048)
QT = 1024              # query tile (psum-sized)
EPS = 1e-5
SCALE = float(MID) ** -0.5
MB = 128               # key block
NMB = N // MB          # 64 key blocks

USE_F32R = True       # fp32r matmuls: 1 cyc/row (vs 4 for fp32) when free>=256

_cache = {}


def _build(loop_r=1):
    import concourse.bacc as bacc
    import concourse.tile as tile
    from concourse import mybir

    f32 = mybir.dt.float32
    f32r = mybir.dt.float32r
    bf16 = mybir.dt.bfloat16
    AF = mybir.ActivationFunctionType
    ALU = mybir.AluOpType

    fmm = f32r if USE_F32R else f32

    nc = bacc.Bacc(trn_type="TRN2", target_bir_lowering=False, debug=False)

    xb = nc.dram_tensor("xb", [C, N], f32, kind="ExternalInput").ap()
    wfT = nc.dram_tensor("wfT", [2, 128, MID], f32, kind="ExternalInput").ap()
    wgT = nc.dram_tensor("wgT", [2, 128, MID], f32, kind="ExternalInput").ap()
    whT = nc.dram_tensor("whT", [2, 128, MID], bf16, kind="ExternalInput").ap()
    wvT = nc.dram_tensor("wvT", [MID, C], f32, kind="ExternalInput").ap()
    bf = nc.dram_tensor("bf", [MID, 1], f32, kind="ExternalInput").ap()
    bg = nc.dram_tensor("bg", [MID, 1], f32, kind="ExternalInput").ap()
    bo = nc.dram_tensor("bo", [C, 1], f32, kind="ExternalInput").ap()
    out = nc.dram_tensor("out", [C, QC], f32, kind="ExternalOutput").ap()

    with tile.TileContext(nc) as tc:
        with (
            tc.tile_pool(name="consts", bufs=1) as consts,
            tc.tile_pool(name="xpool", bufs=1) as xpool,
            tc.tile_pool(name="proj", bufs=1) as proj,
            tc.tile_pool(name="ppool", bufs=3) as ppool,
            tc.tile_pool(name="zpool", bufs=2) as zpool,
            tc.tile_pool(name="opool", bufs=4) as opool,
            tc.tile_pool(name="ps_st", bufs=2, space="PSUM") as ps_st,
            tc.tile_pool(name="ps_acc", bufs=1, space="PSUM") as ps_acc,
            tc.tile_pool(name="ps_gen", bufs=2, space="PSUM") as ps_gen,
        ):
            import contextlib
            loop_ctx = (tc.For_i(0, loop_r, 1) if loop_r > 1
                        else contextlib.nullcontext())
            with loop_ctx:
                # ---- constants + x, interleaved on the SP and Pool DMA
                # queues (ACT's queue must stay clear: its in-order stream
                # would stall the exp pipeline behind any DMA we put there).
                wf_t = consts.tile([128, 2, MID], fmm)
                wg_t = consts.tile([128, 2, MID], fmm)
                wh_t = consts.tile([128, 2, MID], bf16)
                wv_t = consts.tile([MID, C], fmm)
                bf_t = consts.tile([MID, 1], f32)
                bg_t = consts.tile([MID, 1], f32)
                bo_t = consts.tile([128, C // 128, 1], f32)
                scratch = consts.tile([1, 1], f32)

                x_t = xpool.tile([128, 2, N], fmm)
                x_bf = xpool.tile([128, 2, N], bf16)   # h-projection operand
                g_t = proj.tile([MID, N], fmm)
                f_t = proj.tile([MID, QC], fmm)
                hTo = proj.tile([128, NMB, MID + 1], fmm)  # [m, block, c|1]
                pass  # ones column emitted below (needs wf_t DMA'd first)
                xb_r = xb.bitcast(fmm).rearrange("(k p) n -> k p n", p=128)
                CH = [(0, 512), (512, 1024), (1024, 2048), (2048, 4096),
                      (4096, 6144), (6144, 8192)]

                def xdma(q, k, ci):
                    lo, hi = CH[ci]
                    q.dma_start(out=x_t[:, k, lo:hi], in_=xb_r[k, :, lo:hi])

                # SP queue: f weights, first x k=0 chunk, then the rest (c0
                # first: the whole qi0 exp stream only needs f cols 0:1024)
                nc.sync.dma_start(out=wf_t, in_=wfT.rearrange("k p m -> p k m").bitcast(fmm))
                nc.sync.dma_start(out=bf_t, in_=bf)
                xdma(nc.sync, 0, 0)
                nc.sync.dma_start(out=bg_t, in_=bg)
                for ci in range(1, 6):
                    xdma(nc.sync, 0, ci)
                # Pool queue: h/g weights + x k=1 chunks; wv/bo (needed only
                # ~90us in) go after the early chunks
                nc.gpsimd.dma_start(out=wh_t, in_=whT.rearrange("k p m -> p k m"))
                nc.gpsimd.dma_start(out=wg_t, in_=wgT.rearrange("k p m -> p k m").bitcast(fmm))
                xdma(nc.gpsimd, 1, 0)
                xdma(nc.gpsimd, 1, 1)
                xdma(nc.gpsimd, 1, 2)
                nc.gpsimd.dma_start(out=wv_t, in_=wvT.bitcast(fmm))
                nc.gpsimd.dma_start(out=bo_t, in_=bo.rearrange("(o p) x -> p o x", p=128))
                for ci in range(3, 6):
                    xdma(nc.gpsimd, 1, ci)

                cast_done = {0, 1}   # first-kilobyte h-blocks use fp32r x:
                # casting it would occupy DVE exactly when the critical f/g
                # epilogues need it, and PE has slack at the start anyway

                def cast_chunk(ci):
                    # bf16 copy of x chunk for the h projection (DVE)
                    if ci in cast_done:
                        return
                    cast_done.add(ci)
                    lo, hi = CH[ci]
                    for k in range(2):
                        nc.vector.tensor_copy(
                            out=x_bf[:, k, lo:hi],
                            in_=x_t[:, k, lo:hi].bitcast(f32))

                # spin PE up to full p-state before the first real matmul:
                # ~8 dummy matmuls chained from wf_t (lands ~1.4us) keep PE
                # continuously busy past the 3us ramp threshold
                wup = ps_gen.tile([128, 512], f32, tag="pg")
                wsrc = wf_t[:, 0, :]
                for i in range(12):
                    nc.tensor.matmul(wup[0:MID, 0:MID], wsrc, wsrc,
                                     start=True, stop=True,
                                     skip_group_check=True)

                # preload the Exp activation table while DMAs run
                nc.scalar.activation(scratch, bf_t[0:1, :], AF.Exp,
                                     bias=0.0, scale=1.0)
                # ones column for the softmax denominator: early ACT copy
                # (scale=0, bias=1); memset's strided AP fails the ISA check
                # on both Pool and DVE, and anything queued later would stall
                # the very first PV
                nc.scalar.activation(hTo[:, :, MID], wf_t[:, 0, :],
                                     AF.Copy, bias=1.0, scale=0.0)

                # No priming matmuls needed: every fused matmul group below
                # waits on at most one un-absorbed DMA semaphore (k=0 chunks
                # and f/g weights share the SP queue; k=1 chunks and h/v
                # weights share the Pool queue), and a wait on a later
                # transfer of a queue absorbs all earlier ones.

                # ---- projection helpers (DVE epilogues keep ACT exp-only) ----
                def f_tiles(lo_tile, n_tiles):
                    for n in range(lo_tile, lo_tile + n_tiles):
                        sl = slice(n * 512, (n + 1) * 512)
                        pf = ps_gen.tile([MID, 512], f32, tag="pg")
                        for k in range(2):
                            nc.tensor.matmul(pf, wf_t[:, k, :], x_t[:, k, sl],
                                             start=(k == 0), stop=(k == 1))
                        nc.vector.tensor_scalar(f_t[:, sl], pf, bf_t, 0.0,
                                                ALU.add, ALU.max)

                def g_tile(n):
                    sl = slice(n * 512, (n + 1) * 512)
                    pg = ps_gen.tile([MID, 512], f32, tag="pg")
                    for k in range(2):
                        nc.tensor.matmul(pg, wg_t[:, k, :], x_t[:, k, sl],
                                         start=(k == 0), stop=(k == 1))
                    nc.vector.tensor_scalar(g_t[:, sl], pg, bg_t, 0.0,
                                            ALU.add, ALU.max)

                wh_r = consts.tile([128, 2, MID], fmm)
                nc.vector.tensor_copy(out=wh_r, in_=wh_t)   # f32r copy of wh

                def h_block(mb):
                    ph = ps_gen.tile([128, MID], f32, tag="pg")
                    msl = slice(mb * MB, (mb + 1) * MB)
                    use_bf = mb * MB >= 1024
                    for k in range(2):
                        if use_bf:
                            nc.tensor.matmul(ph, x_bf[:, k, msl], wh_t[:, k, :],
                                             start=(k == 0), stop=(k == 1))
                        else:
                            nc.tensor.matmul(ph, x_t[:, k, msl], wh_r[:, k, :],
                                             start=(k == 0), stop=(k == 1))
                    nc.vector.tensor_copy(out=hTo[:, mb, 0:MID], in_=ph)

                def attn_block(qi, mb, o_ps):
                    msl = slice(mb * MB, (mb + 1) * MB)
                    st = ps_st.tile([128, QT], f32, tag="st")
                    for h in range(QT // 512):
                        fs = slice(qi * QT + h * 512, qi * QT + (h + 1) * 512)
                        nc.tensor.matmul(st[:, h * 512:(h + 1) * 512],
                                         g_t[:, msl], f_t[:, fs],
                                         start=True, stop=True)
                    p_t = ppool.tile([128, QT], fmm, tag="p")
                    nc.scalar.activation(p_t, st, AF.Exp, scale=SCALE)
                    for h in range(QT // 512):
                        hs = slice(h * 512, (h + 1) * 512)
                        nc.tensor.matmul(o_ps[:, hs], hTo[:, mb, :],
                                         p_t[:, hs],
                                         start=(mb == 0), stop=(mb == NMB - 1))

                def normalize_out(qi, o_ps):
                    # z = O / denom; out = Wv z + bo + x (fused on Pool);
                    # per 512-col half so the tail pipelines. Both recips
                    # first, then both muls: frees o_ps (WAR for the next
                    # qtile's PVs) as early as possible.
                    rds, rbs, zs = [], [], []
                    for h in range(QT // 512):
                        hs = slice(h * 512, (h + 1) * 512)
                        rd = zpool.tile([1, 512], f32, tag="rd")
                        nc.vector.reciprocal(rd, o_ps[MID:MID + 1, hs])
                        rds.append(rd)
                    for h in range(QT // 512):
                        rb = zpool.tile([MID, 512], f32, tag="rb")
                        nc.gpsimd.partition_broadcast(rb, rds[h])
                        rbs.append(rb)
                    for h in range(QT // 512):
                        hs = slice(h * 512, (h + 1) * 512)
                        z_t = zpool.tile([MID, 512], fmm, tag="z")
                        nc.vector.tensor_mul(z_t, o_ps[0:MID, hs], rbs[h])
                        zs.append(z_t)
                    for h in range(QT // 512):
                        z_t = zs[h]
                        for oh in range(C // 128):
                            osl = slice(oh * 128, (oh + 1) * 128)
                            po = ps_gen.tile([128, 512], f32, tag="pg")
                            nc.tensor.matmul(po, wv_t[:, osl], z_t,
                                             start=True, stop=True)
                            o_sb = opool.tile([128, 512], f32, tag="ob")
                            qs = slice(qi * QT + h * 512, qi * QT + (h + 1) * 512)
                            # DVE (GPSIMD cannot access PSUM on real HW)
                            nc.vector.scalar_tensor_tensor(
                                o_sb, po, bo_t[:, oh, :],
                                x_t[:, oh, qs].bitcast(f32),
                                ALU.add, ALU.add)
                            # ACT's queue is free once the last qtile drains;
                            # use it to parallelize the output DMA
                            oq = nc.scalar if (qi == QC // QT - 1 and h == 1) \
                                else nc.sync
                            oq.dma_start(out=out.rearrange(
                                "(o p) n -> o p n", p=128)[oh, :, qs], in_=o_sb)

                # ---- qi0 attention with just-in-time projections: emission
                # order is the scheduler's priority order, so proj work is
                # spread per-block instead of front-loading ~30us of PE ----
                o_ps0 = ps_acc.tile([MID + 1, QT], f32, tag="acc")
                f_tiles(0, 2)          # f cols 0:1024 — all qi0 needs
                for mb in range(NMB):
                    if mb % 4 == 0:
                        g_tile(mb // 4)
                    ci = next(i for i, (lo, hi) in enumerate(CH)
                              if lo <= mb * MB < hi)
                    cast_chunk(ci)
                    h_block(mb)
                    attn_block(0, mb, o_ps0)
                f_tiles(2, 2)          # f cols 1024:2048 for qi1

                # drain qi0 (frees o_ps for qi1's PVs) while qi1's S^T/exp
                # stream continues uninterrupted on PE/ACT
                normalize_out(0, o_ps0)
                o_ps1 = ps_acc.tile([MID + 1, QT], f32, tag="acc")
                for mb in range(NMB):
                    attn_block(1, mb, o_ps1)
                normalize_out(1, o_ps1)

    nc.compile()
    return nc


def _prep_inputs(inputs):
    f32 = np.float32
    x = np.asarray(inputs["x"], f32).reshape(B, C, N)

    def fold(W, b, gam, bet, m, v):
        inv = np.asarray(gam, f32) / np.sqrt(np.asarray(v, f32) + EPS)
        We = np.asarray(W, f32) * inv[:, None]
        be = np.asarray(b, f32) * inv + np.asarray(bet, f32) - np.asarray(m, f32) * inv
        return We, be

    Wf, bfe = fold(inputs["Wf"], inputs["bf"], inputs["gamf"], inputs["betf"],
                   inputs["mf"], inputs["vf"])
    Wg, bge = fold(inputs["Wg"], inputs["bg"], inputs["gamg"], inputs["betg"],
                   inputs["mg"], inputs["vg"])
    Wh = np.asarray(inputs["Wh"], f32)
    Wv = np.asarray(inputs["Wv"], f32)
    bo = Wv @ np.asarray(inputs["bh"], f32) + np.asarray(inputs["bv"], f32)

    import ml_dtypes
    wfT = np.ascontiguousarray(Wf.T.reshape(2, 128, MID))
    wgT = np.ascontiguousarray(Wg.T.reshape(2, 128, MID))
    whT = np.ascontiguousarray(Wh.T.reshape(2, 128, MID)
                               .astype(ml_dtypes.bfloat16))
    wvT = np.ascontiguousarray(Wv.T)

    in_maps = []
    for core in range(NCORES):
        b, qc = divmod(core, 4)
        qoff = qc * QC
        # rotate this core's queries to the front (keys are a set; softmax
        # and the PV sum are invariant to key order)
        xrot = np.concatenate([x[b][:, qoff:], x[b][:, :qoff]], axis=1)
        in_maps.append({
            "xb": np.ascontiguousarray(xrot),
            "wfT": wfT, "wgT": wgT, "whT": whT, "wvT": wvT,
            "bf": bfe.reshape(MID, 1).copy(),
            "bg": bge.reshape(MID, 1).copy(),
            "bo": bo.reshape(C, 1).copy(),
        })
    return in_maps


def _run(inputs, trace=False, **kw):
    from concourse.bass_utils import run_bass_kernel_spmd

    if "nc" not in _cache:
        _cache["nc"] = _build()
    in_maps = _prep_inputs(inputs)
    br = run_bass_kernel_spmd(_cache["nc"], in_maps, list(range(NCORES)),
                              trace=trace, **kw)
    out = np.empty((B, C, N), np.float32)
    for core in range(NCORES):
        b, qc = divmod(core, 4)
        out[b][:, qc * QC:(qc + 1) * QC] = br.results[core]["out"]
    return out.reshape(B, C, 8, 32, 32), br


def kernel(**inputs):
    out, _ = _run(inputs)
    return out
